# revision 20
# baseline (speedup 1.0000x reference)
"""Trainium2 Bass kernel for ConditionCrossAttention2D (v3 - transposed PV).

Reference computation (per batch item b, with n = H*W spatial positions):
    q = Wq @ cond + bq            # [Ck, n]
    k = Wk @ feat + bk            # [Ck, n]
    v = Wv @ feat + bv            # [C, n]
    energy[i, j] = sum_ck q[ck, i] * k[ck, j]
    attn = softmax_j(energy)
    out[c, i] = sum_j v[c, j] * attn[i, j]
    result = gamma * out + feat

Sharding: 8 cores = (batch b in 0..3) x (query-half h in 0..1); each core
handles 2048 queries x 4096 keys with no cross-core communication.

Per-core design (v3):
  - energy is computed with QUERIES on partitions: e[i, j] via stationary
    q-tile [32ck x 128i] and moving k (4x-replicated row groups cover
    1024 keys per pass).  The exp then runs with j on the free axis, so
    the ACT engine's accum_out register yields the softmax denominators
    (pre-quantization fp32 row sums) for free, in the layout that makes
    the reciprocal a cheap 128-lane op.  No ones-column, no PE
    transposes, no Scalar-engine finalize work: ACT does exp only.
  - attn [i, j] fp8 is transposed to attnT [j-pairs, i] by the DMA xbar
    transpose engine (fp8 byte-pairs viewed as u16), one issue per
    (query-tile, 1024-key group) on the otherwise idle Sync queue.  The
    transposed tile IS the DoubleRow moving operand of the PV matmul.
  - PV runs with vT as the stationary operand: out[c, i] accumulates in
    PSUM over all 16 key-pair blocks with N=512 moving streams
    (LDWEIGHTS 213ns hides under the 241ns stream; the baseline's
    attnT-stationary form was LDWEIGHTS-bound at ~282ns per 272-col
    matmul).  Output lands directly in the [C, NL] layout of out_d.
  - normalization + residual fuse into the PV evacuation: the per-chunk
    reciprocals hop through DRAM to become a partition-replicated
    [128, 512] operand, then two DVE tensor_tensor passes (mult by
    rcp, add residual) produce the store tile.
  - biases fold into the projection matmul chains; Wv/bv are pre-scaled
    by gamma on the host; feat/cond stream in as fp8e4m3.
  - input DMAs split across Sync (weights+cond), GpSimd (feat, fres,
    rcp hops, output stores) and Scalar (small consts) queues.
"""

import os
from contextlib import ExitStack

import numpy as np

import concourse.bass as bass
import concourse.tile as tile
from concourse import mybir
from concourse.bass_utils import run_bass_kernel_spmd

B, C, CK, H, W = 4, 256, 32, 64, 64
N = H * W            # 4096 spatial positions
NCORES = 8
NL = N // 2          # 2048 queries per core
P = 128
NIT = NL // P        # 16 query tiles per core
NJG = N // 1024      # 4 key groups of 1024
NJPB = N // 256      # 16 key-pair blocks of 128 u16 pairs
NCH = 4              # query chunks of 512 (4 it tiles) for PV psum
LAG = 14             # pv pair-steps of pipeline lag behind the exp steps
F32 = mybir.dt.float32
BF16 = mybir.dt.bfloat16
F8 = mybir.dt.float8e4
U16 = mybir.dt.uint16
EXP = mybir.ActivationFunctionType.Exp
ADD = mybir.AluOpType.add
MULT = mybir.AluOpType.mult

STAGE = int(os.environ.get('KSTAGE', '3'))
LAST_EXEC_TIME_NS = None
LAST_TRACE = None

ts = bass.ts


def _emit(tc, ctx):
    nc = tc.nc

    feat_d = nc.declare_dram_parameter("feat", [C, N], F8, isOutput=False)
    cond_d = nc.declare_dram_parameter("cond", [C, NL], F8, isOutput=False)
    fres_d = nc.declare_dram_parameter("fres", [C, NL], F32, isOutput=False)
    wqt_d = nc.declare_dram_parameter("Wqt", [P, 2 * 4 * CK], BF16, isOutput=False)
    wkt_d = nc.declare_dram_parameter("Wkt", [P, 2 * 4 * CK], BF16, isOutput=False)
    wvt_d = nc.declare_dram_parameter("Wvt", [P, 2 * C], BF16, isOutput=False)
    bq_d = nc.declare_dram_parameter("bq", [CK], F32, isOutput=False)
    bk_d = nc.declare_dram_parameter("bk", [CK], F32, isOutput=False)
    bv_d = nc.declare_dram_parameter("bv", [C], F32, isOutput=False)
    rcp_d = nc.declare_dram_parameter("rcpbuf", [NCH * 4 * P], F32, isOutput=True)
    out_d = nc.declare_dram_parameter("out", [C, NL], F32, isOutput=True)

    def bcast_ap(handle, parts, free):
        ap = handle[:]
        return bass.AP(tensor=ap.tensor, offset=ap.offset, ap=[[0, parts], [1, free]])

    consts = ctx.enter_context(tc.tile_pool(name="consts", bufs=1))
    persist = ctx.enter_context(tc.tile_pool(name="persist", bufs=1))
    loads = ctx.enter_context(tc.tile_pool(name="loads", bufs=1))
    attnp = ctx.enter_context(tc.tile_pool(name="attn", bufs=3))
    attnTp = ctx.enter_context(tc.tile_pool(name="attnT", bufs=2))
    rcpbp = ctx.enter_context(tc.tile_pool(name="rcpb", bufs=2))
    finp = ctx.enter_context(tc.tile_pool(name="fin", bufs=4))
    # PSUM (8 banks): energy 2x2 banks, pv 2x2 banks.
    ep = ctx.enter_context(tc.tile_pool(name="ep", bufs=2, space="PSUM"))
    pvp = ctx.enter_context(tc.tile_pool(name="pvps", bufs=2, space="PSUM"))

    # Preload the ACT exp table set during the DMA prologue.  The dummy
    # accum_out also drains the ACT accumulator register, which powers up
    # with garbage and is only reset by a READ_ACCUMULATOR.
    warm = consts.tile([P, 1], F32)
    warm_acc = consts.tile([P, 1], F32)
    nc.gpsimd.memset(warm[:], 0.0)
    nc.scalar.activation(warm[:], warm[:], EXP, accum_out=warm_acc[:])

    # Scratch operands for the PE warm-up / filler matmuls (HAM clock gate).
    wsrc = consts.tile([P, 512], BF16)
    nc.gpsimd.memset(wsrc[:], 0.0)

    def pe_filler(n):
        for _ in range(n):
            wps = pvp.tile([P, 2, 512], F32, tag="pv", name="wps")
            nc.tensor.matmul(wps[:, 0, :], wsrc[:, 0:P], wsrc[:],
                             start=True, stop=True)

    # Transposed weights (bf16): wq_t[p, ct, 32r+ck] = Wq[ck, ct*128+p] x4
    wq_t = consts.tile([P, 2, 4 * CK], BF16)
    wk_t = consts.tile([P, 2, 4 * CK], BF16)
    wv_t = consts.tile([P, 2, C], BF16)

    bv_b = consts.tile([P, C], F32)
    bq_c = consts.tile([4 * CK, 1], F32)
    bk_c = consts.tile([4 * CK, 1], F32)

    # ---- input loads, split across engine DMA queues ----
    nc.sync.dma_start(out=wq_t[:], in_=wqt_d[:, :])
    nc.sync.dma_start(out=wk_t[:], in_=wkt_d[:, :])
    nc.sync.dma_start(out=wv_t[:], in_=wvt_d[:, :])

    def load_chunk(eng, dram, ncols, col0, tag):
        t = loads.tile([P, 2, 512], F8, tag=tag, name="in_f8")
        ap = dram[:, :]
        src = bass.AP(tensor=ap.tensor, offset=col0,
                      ap=[[ncols, P], [P * ncols, 2], [1, 512]])
        eng.dma_start(out=t[:], in_=src)
        return t

    cond_c = [load_chunk(nc.sync, cond_d, NL, icc * 512, f"cond{icc}")
              for icc in range(NL // 512)]

    def rep4_ap(handle):
        ap = handle[:]
        return bass.AP(tensor=ap.tensor, offset=ap.offset,
                       ap=[[0, 4], [1, CK]])

    nc.scalar.dma_start(out=bq_c[:], in_=rep4_ap(bq_d))
    nc.scalar.dma_start(out=bk_c[:], in_=rep4_ap(bk_d))
    nc.scalar.dma_start(out=bv_b[:], in_=bcast_ap(bv_d, P, C))

    feat_c = [load_chunk(nc.gpsimd, feat_d, N, ncc * 512, f"feat{ncc}")
              for ncc in range(N // 512)]

    # PE warm-up bridge (HAM clock gate: ~3.4us to trip to 2.4 GHz).
    pe_filler(21)

    feat_res = persist.tile([P, 2, NL], F32)

    # Projection outputs; partitions 32..63 etc. hold replicas of 0..31.
    q_rep = persist.tile([P, NL], BF16)           # q[ck, i] x4 (stationary)
    k_rep = persist.tile([P, N], BF16)            # k[ck, j] x4 (moving)
    # vT[p, jpb, ko, c] = gamma * v[c, 2*(jpb*128+p)+ko]  (PV stationary)
    vT_sb = persist.tile([P, NJPB, 2, C], F8)
    # exp accum row sums (pre-quantization fp32): denominators
    den_parts = persist.tile([P, NIT, NJG], F32)

    # ---- q/k projections in 512-col chunks through the pv psum pool ----
    def qk_chunk(wt, bias_c, src_chunk, dst, dcol, fill=0):
        ps = pvp.tile([P, 2, 512], F32, tag="pv", name="qk_ps")
        for ct in range(2):
            nc.tensor.matmul(ps[:, 0, :], wt[:, ct, :], src_chunk[:, ct, :],
                             start=(ct == 0), stop=(ct == 1))
        nc.vector.tensor_scalar(dst[:, dcol:dcol + 512], ps[:, 0, :],
                                bias_c[:], None, op0=ADD)
        pe_filler(fill)

    for icc in range(4):
        qk_chunk(wq_t, bq_c, cond_c[icc], q_rep, icc * 512)
    for ncc in range(4):
        qk_chunk(wk_t, bk_c, feat_c[ncc], k_rep, ncc * 512)
    # NOTE: all transient psum users (qk chunks, vt units, fillers) share
    # the pv pool's rotation and MUST be emitted before the first pv_step
    # allocates its long-lived accumulation tile (gs >= LAG).

    # ---- vT projection unit: one (kb, ko) -> vT_sb[:, kb, ko, :] ----
    # stationary: feat columns kb*256 + 2m + ko (stride-2 select), so out
    # partition m holds key 2*(kb*128+m)+ko, matching the xbar transpose
    # pair layout jp = blk*128 + p.
    def vt_unit(kb, ko):
        v_ps = pvp.tile([P, 2, 512], F32, tag="pv", name="v_ps")
        ch, bl = divmod(kb, 2)
        for ct in range(2):
            col0 = bl * 256 + ko
            nc.tensor.matmul(
                v_ps[:, 0, 0:C],
                feat_c[ch][:, ct, col0:bl * 256 + 256:2],
                wv_t[:, ct, :],
                start=(ct == 0), stop=(ct == 1))
        nc.vector.tensor_tensor(vT_sb[:, kb, ko, :],
                                v_ps[:, 0, 0:C], bv_b[:], op=ADD)

    # ---- steady-state structures ----
    attn_tiles = {}     # (it, jg) -> attn fp8 [P, 1024]
    attnT_tiles = {}    # chunk -> [P, NJPB, 4, 128] u16
    pv_tiles = {}       # chunk -> psum [P, 2, 512]
    rcp_bs = {}

    def exp_step(gs):
        # within each 4-it chunk, sweep jg-outer/it-inner so the woven
        # k-chunk projections (emitted at gs 0,2,4,6) precede, in program
        # order, the first energy reads of their key columns (gs >= 4*jg).
        ch, r = divmod(gs, 16)
        jg, itl = divmod(r, 4)
        it = 4 * ch + itl
        # 2 concurrent row-group matmuls, one psum bank each (4 row groups
        # with two 256-col dsts per bank hangs the HW - bank write port).
        # jg parity alternates the q/k replica pair to spread SBUF reads.
        e_ps = ep.tile([P, 1024], F32, tag="e", name="e_ps")
        for sr in range(2):
            rr = 2 * (jg % 2) + sr
            j0 = jg * 1024 + sr * 512
            nc.tensor.matmul(
                e_ps[:, ts(sr, 512)],
                q_rep[ts(rr, CK), ts(it, P)],
                k_rep[ts(rr, CK), j0:j0 + 512],
                start=True, stop=True, tile_position=(32 * rr, 0))
        at = attnp.tile([P, 1024], F8, tag="attn", name="attn")
        attn_tiles[(it, jg)] = at
        nc.scalar.activation(at[:], e_ps[:], EXP,
                             accum_out=den_parts[:, it, jg:jg + 1])
        # DMA xbar transpose: attn [i, j] -> attnT u16 [jp, i] on sync queue
        # itl-major dst layout => contiguous 1KB dst runs per partition (a
        # jpb-major dst fragments the xbar DMA into 256B packets; measured
        # 52 GB/s aggregate, which paced the entire kernel).
        if (itl, jg) == (0, 0):
            attnT_tiles[ch] = attnTp.tile([P, 4, NJPB, P], U16,
                                          tag="attnT", name="attnT")
        if STAGE >= 1:
            nc.sync.dma_start_transpose(
                attnT_tiles[ch][:, itl, 4 * jg:4 * jg + 4, :],
                at[:].bitcast(U16))

    def pv_step(s):
        ch, jpb = divmod(s, NJPB)
        if jpb == 0:
            pv_tiles[ch] = pvp.tile([P, 2, 512], F32, tag="pv", name="pv")
        pv_t = pv_tiles[ch]
        mov = attnT_tiles[ch][:, :, jpb, :].bitcast(F8).rearrange(
            "p a (x k) -> p k a x", k=2)
        for cb in range(2):
            nc.tensor.matmul(
                pv_t[:, cb, :],
                vT_sb[:, jpb, :, ts(cb, P)],
                mov,
                start=(jpb == 0), stop=(jpb == NJPB - 1),
                perf_mode=mybir.MatmulPerfMode.DoubleRow)
        if jpb == NJPB - 1:
            finalize_chunk(ch)

    def finalize_chunk(ch):
        # denominators: reduce the 4 per-group accums, reciprocal, then
        # hop through DRAM to become a partition-replicated [P, 512]
        # free-axis operand for the normalize multiply.
        den_r = finp.tile([P, 4], F32, tag="denr")
        nc.vector.tensor_reduce(den_r[:], den_parts[:, 4 * ch:4 * ch + 4, :],
                                axis=mybir.AxisListType.X, op=ADD)
        rcp = finp.tile([P, 4], F32, tag="rcp")
        nc.vector.reciprocal(rcp[:], den_r[:])
        # hop1 writes DRAM already in output-column order (itl*128 + p) so
        # hop2 is a plain contiguous partition-broadcast read.
        rb = rcpbp.tile([P, 512], F32, tag="rcpb", name="rcp_b")
        rcp_bs[ch] = rb
        if STAGE >= 3:
            # hop through DRAM to reorder [p, itl] -> column order itl*128+p,
            # land in one partition, then gpsimd broadcasts to all 128.
            # (a stride-0-partition DRAM->SBUF broadcast DMA degenerates to
            # 4-byte packets and takes ~17us - measured.)
            rda = rcp_d[:]
            h1dst = bass.AP(tensor=rda.tensor, offset=ch * 512,
                            ap=[[1, P], [P, 4]])
            i1 = nc.gpsimd.dma_start(out=h1dst, in_=rcp[:])
            src = bass.AP(tensor=rda.tensor, offset=ch * 512,
                          ap=[[0, P], [1, 512]])
            i2 = nc.sync.dma_start(out=rb[:], in_=src)
            # explicit hop1 -> hop2 ordering: Tile does not track the DRAM
            # scratch tensor, and the two DMAs are on different queues.
            from concourse.instruction_name_ordered_set import (
                InstructionNameOrderedSet,
            )
            deps = InstructionNameOrderedSet()
            deps.add(i1.ins.name)
            i2.ins.add_sync_dependencies_from(deps)
        else:
            nc.gpsimd.memset(rb[:], 1.0)
        for cb in range(2):
            st = finp.tile([P, 512], F32, tag=f"st{cb}")
            nc.vector.tensor_tensor(st[:], pv_tiles[ch][:, cb, :], rb[:],
                                    op=MULT)
            so = finp.tile([P, 512], F32, tag=f"so{cb}")
            nc.vector.tensor_tensor(so[:], st[:],
                                    feat_res[:, cb, ts(ch, 512)], op=ADD)
            nc.gpsimd.dma_start(out=out_d[ts(cb, P), ts(ch, 512)], in_=so[:])

    # ---- software pipeline ----
    # exp steps 0..63 (it-outer, jg-inner); pv pair-steps lag by LAG.
    nsteps = NIT * NJG
    vt_queue = [(kb, ko) for kb in range(NJPB) for ko in range(2)]
    for u in range(6):
        vt_unit(*vt_queue[u])
    vq = 6
    for gs in range(nsteps + LAG):
        if gs == 20:
            for cb in range(2):
                nc.gpsimd.dma_start(out=feat_res[:, cb, :],
                                    in_=fres_d[ts(cb, P), :])
        if gs < nsteps:
            exp_step(gs)
            # remaining k chunks + v units woven into the pre-PV steps
            # (they cycle the pv psum pool - see NOTE above)
            if gs < 8 and gs % 2 == 0:
                ncc = 4 + gs // 2
                qk_chunk(wk_t, bk_c, feat_c[ncc], k_rep, ncc * 512)
            if gs < LAG - 1:
                take = min(2, len(vt_queue) - vq)
                for _ in range(take):
                    vt_unit(*vt_queue[vq])
                    vq += 1
        if STAGE >= 2 and gs >= LAG and gs - LAG < nsteps:
            pv_step(gs - LAG)
    assert vq == len(vt_queue), f"vt units left over: {vq}"
    if os.environ.get('KDUMPDEN', '0') == '1':
        dd = bass.AP(tensor=rcp_d[:].tensor, offset=0, ap=[[16, P], [1, 16]])
        nc.gpsimd.dma_start(out=dd, in_=den_parts[:, 0:4, :])
    if STAGE < 2:
        # debug: residual-only output (correct for gamma=0) + rcpbuf write
        for ch in range(NCH):
            for cb in range(2):
                nc.gpsimd.dma_start(out=out_d[ts(cb, P), ts(ch, 512)],
                                    in_=feat_res[:, cb, ts(ch, 512)])
        nc.gpsimd.dma_start(out=rcp_d[0:512], in_=den_parts[:, 0, :])


def _split_ctrl_waits(nc, cap=1):
    """Walrus in this image allows only ONE sync-wait command per
    instruction; Tile emits several on phase-boundary instructions (and one
    per live semaphore on the kernel-tail drain). Splitting the excess waits
    onto preceding same-engine NoOps is semantically identical (engine
    sequencers execute in order, so waiting on A then B == waiting on both)."""
    for fn in nc.m.functions:
        for bb in fn.blocks:
            insts = bb.instructions
            out = []
            changed = False
            for ins in insts:
                si = ins.sync_info
                if si is not None and si.on_wait and len(si.on_wait) > cap:
                    waits = list(si.on_wait)
                    for i, w in enumerate(waits[:-cap]):
                        nop = mybir.InstNoOp(
                            name=f"{ins.name}-w{i}",
                            engine=ins.engine,
                            ins=[], outs=[],
                            sync_info=mybir.SyncInfo(on_wait=[w], on_update=[]),
                        )
                        if hasattr(nc, "register_instruction"):
                            nc.register_instruction(nop, overwrite=True)
                        out.append(nop)
                    ins.sync_info = mybir.SyncInfo(
                        on_wait=waits[-cap:], on_update=list(si.on_update))
                    changed = True
                out.append(ins)
            if changed:
                insts[:] = out


def build_nc():
    nc = bass.Bass()
    with tile.TileContext(nc) as tc, ExitStack() as ctx:
        _emit(tc, ctx)
    _split_ctrl_waits(nc)
    return nc


def _prep_wt(w, rep):
    # [K, C] fp32 -> [128, 2, K*rep] bf16 with w_t[p, ct, r*K+k] = w[k, ct*128+p]
    import ml_dtypes
    wt = np.asarray(w, np.float32).T.reshape(2, P, -1)       # [ct, p, K]
    wt = np.transpose(wt, (1, 0, 2))                          # [p, ct, K]
    wt = np.tile(wt, (1, 1, rep))
    return np.ascontiguousarray(wt.reshape(P, -1).astype(ml_dtypes.bfloat16))


def make_in_maps(features, conditions, Wq, bq, Wk, bk, Wv, bv, gamma):
    import ml_dtypes
    feat = np.ascontiguousarray(np.asarray(features, np.float32).reshape(B, C, N))
    cond = np.ascontiguousarray(np.asarray(conditions, np.float32).reshape(B, C, N))
    feat_f8 = feat.astype(ml_dtypes.float8_e4m3fn)
    cond_f8 = cond.astype(ml_dtypes.float8_e4m3fn)
    g = np.float32(np.asarray(gamma, np.float32).reshape(()))
    wqt = _prep_wt(Wq, 4)
    wkt = _prep_wt(Wk, 4)
    wvt = _prep_wt(np.asarray(Wv, np.float32) * g, 1)
    bq_ = np.ascontiguousarray(np.asarray(bq, np.float32))
    bk_ = np.ascontiguousarray(np.asarray(bk, np.float32))
    bv_ = np.ascontiguousarray(np.asarray(bv, np.float32) * g)
    in_maps = []
    for core in range(NCORES):
        b, h = divmod(core, 2)
        n0 = h * NL
        in_maps.append({
            "feat": feat_f8[b],
            "cond": np.ascontiguousarray(cond_f8[b][:, n0:n0 + NL]),
            "fres": np.ascontiguousarray(feat[b][:, n0:n0 + NL]),
            "Wqt": wqt, "Wkt": wkt, "Wvt": wvt,
            "bq": bq_, "bk": bk_, "bv": bv_,
        })
    return in_maps


def kernel(features, conditions, Wq, bq, Wk, bk, Wv, bv, gamma):
    global LAST_EXEC_TIME_NS, LAST_TRACE
    in_maps = make_in_maps(features, conditions, Wq, bq, Wk, bk, Wv, bv, gamma)
    nc = build_nc()
    trace = os.environ.get("BASS_KERNEL_TRACE", "0") == "1"
    res = run_bass_kernel_spmd(nc, in_maps, list(range(NCORES)), trace=trace)
    LAST_EXEC_TIME_NS = res.exec_time_ns
    LAST_TRACE = res.instructions_and_trace
    out = np.empty((B, C, N), np.float32)
    for core in range(NCORES):
        b, h = divmod(core, 2)
        out[b][:, h * NL:(h + 1) * NL] = res.results[core]["out"]
    return out.reshape(B, C, H, W)


# revision 24
# speedup vs baseline: 1.6276x; 1.6276x over previous
"""Trainium2 Bass kernel for ConditionCrossAttention2D (v4 - direct-paired PV).

Reference computation (per batch item b, with n = H*W spatial positions):
    q = Wq @ cond + bq            # [Ck, n]
    k = Wk @ feat + bk            # [Ck, n]
    v = Wv @ feat + bv            # [C, n]
    energy[i, j] = sum_ck q[ck, i] * k[ck, j]
    attn = softmax_j(energy)
    out[c, i] = sum_j v[c, j] * attn[i, j]
    result = gamma * out + feat
Sharding: 8 cores = (batch b) x (query-half h); each core: 2048 q x 4096 k.

Per-core design (v4):
  - energy e_T[j, i] with KEYS on partitions, where the stationary k
    operand selects stride-2 key columns so psum partition p of plane ko
    holds key 2*(jpb*128+p)+ko.  The exp's strided fp8 output AP then
    writes attnT in DoubleRow pair-interleaved layout DIRECTLY - no
    on-chip transpose anywhere (a DMA-xbar transpose of the 8.4MB attn
    measured only 52 GB/s aggregate and paced the whole kernel; PE
    transposes would double TensorE time).
  - PV runs with vT as stationary and attnT as the N=512 DR moving
    stream: out[c, i] accumulates over the 16 key-pair blocks straight
    into the [C, NL] output layout (no finalize transposes).
  - softmax denominators: one M=1 ones-stationary DR matmul per
    (chunk, key-pair block) accumulates den[1, 512] in psum; at chunk
    end it is copied out, PE-transposed in 128-col blocks to get
    queries onto partitions, reciprocal'd on DVE, and round-trips
    through DRAM + a hwdge broadcast DMA to become the [128, 512]
    free-axis operand of the normalize multiply.  (The ACT accum_out
    path would need queries on partitions during exp - wrong axis here.)
  - normalization + residual fuse into the PV psum evacuation (two DVE
    tensor_tensor passes), and the store is the natural [C, NL] layout.
  - ScalarE does exp ONLY: 64 x N=1024 ACTIVATEs are the target
    critical path (~73us).
  - biases fold into the projection matmuls; Wv/bv pre-scaled by gamma
    on the host; feat/cond stream in as fp8e4m3.
"""

import os
from contextlib import ExitStack

import numpy as np

import concourse.bass as bass
import concourse.tile as tile
from concourse import mybir
from concourse.bass_utils import run_bass_kernel_spmd
from concourse.masks import make_identity

B, C, CK, H, W = 4, 256, 32, 64, 64
N = H * W            # 4096 spatial positions
NCORES = 8
NL = N // 2          # 2048 queries per core
P = 128
NJPB = N // 256      # 16 key-pair blocks (256 keys each)
NCH = NL // 512      # 4 query chunks of 512
LAG = 3              # pv steps of pipeline lag behind the exp steps
F32 = mybir.dt.float32
BF16 = mybir.dt.bfloat16
F8 = mybir.dt.float8e4
EXP = mybir.ActivationFunctionType.Exp
ADD = mybir.AluOpType.add
MULT = mybir.AluOpType.mult

LAST_EXEC_TIME_NS = None
LAST_TRACE = None

ts = bass.ts


def _emit(tc, ctx):
    nc = tc.nc

    feat_d = nc.declare_dram_parameter("feat", [C, N], F8, isOutput=False)
    cond_d = nc.declare_dram_parameter("cond", [C, NL], F8, isOutput=False)
    fres_d = nc.declare_dram_parameter("fres", [C, NL], F32, isOutput=False)
    wqt_d = nc.declare_dram_parameter("Wqt", [P, 2 * 4 * CK], BF16, isOutput=False)
    wkt_d = nc.declare_dram_parameter("Wkt", [P, 2 * 4 * CK], BF16, isOutput=False)
    wvt_d = nc.declare_dram_parameter("Wvt", [P, 2 * C], BF16, isOutput=False)
    bq_d = nc.declare_dram_parameter("bq", [CK], F32, isOutput=False)
    bk_d = nc.declare_dram_parameter("bk", [CK], F32, isOutput=False)
    bv_d = nc.declare_dram_parameter("bv", [C], F32, isOutput=False)
    rcp_d = nc.declare_dram_parameter("rcpbuf", [NCH * 4 * P], F32, isOutput=True)
    out_d = nc.declare_dram_parameter("out", [C, NL], F32, isOutput=True)

    def bcast_ap(handle, parts, free):
        ap = handle[:]
        return bass.AP(tensor=ap.tensor, offset=ap.offset, ap=[[0, parts], [1, free]])

    consts = ctx.enter_context(tc.tile_pool(name="consts", bufs=1))
    persist = ctx.enter_context(tc.tile_pool(name="persist", bufs=1))
    loads = ctx.enter_context(tc.tile_pool(name="loads", bufs=1))
    attnTp = ctx.enter_context(tc.tile_pool(name="attnT", bufs=2))
    rcpbp = ctx.enter_context(tc.tile_pool(name="rcpb", bufs=2))
    finp = ctx.enter_context(tc.tile_pool(name="fin", bufs=4))
    # PSUM (8 banks): energy 2x2 banks, pv-set rotation over 4 banks
    # (3 one-bank tiles per chunk: cb0, cb1, den).
    ep = ctx.enter_context(tc.tile_pool(name="ep", bufs=2, space="PSUM"))
    pvp = ctx.enter_context(tc.tile_pool(name="pvps", bufs=4, space="PSUM"))

    # Preload the ACT exp table set during the DMA prologue.
    warm = consts.tile([P, 1], F32)
    nc.gpsimd.memset(warm[:], 0.0)
    nc.scalar.activation(warm[:], warm[:], EXP)

    # Scratch operands for the PE warm-up / filler matmuls (HAM clock gate).
    wsrc = consts.tile([P, 512], BF16)
    nc.gpsimd.memset(wsrc[:], 0.0)

    ident = consts.tile([P, P], F32)
    make_identity(nc, ident)

    # fp8 ones for the denominator DR matmuls (stationary [128, 2, 1],
    # ko step 16 bytes to satisfy the DoubleRow AP constraint).
    ones8 = consts.tile([P, 2, 16], F8)
    nc.vector.memset(ones8[:], 1.0)

    def pe_filler(n):
        for _ in range(n):
            wps = pvp.tile([P, 512], F32, tag="pv", name="wps")
            nc.tensor.matmul(wps[:], wsrc[:, 0:P], wsrc[:],
                             start=True, stop=True)

    # Transposed weights (bf16): wq_t[p, ct, 32r+ck] = Wq[ck, ct*128+p] x4
    wq_t = consts.tile([P, 2, 4 * CK], BF16)
    wk_t = consts.tile([P, 2, 4 * CK], BF16)
    wv_t = consts.tile([P, 2, C], BF16)

    bv_b = consts.tile([P, C], F32)
    bq_c = consts.tile([4 * CK, 1], F32)
    bk_c = consts.tile([4 * CK, 1], F32)

    # ---- input loads, split across engine DMA queues ----
    nc.sync.dma_start(out=wq_t[:], in_=wqt_d[:, :])
    nc.sync.dma_start(out=wk_t[:], in_=wkt_d[:, :])
    nc.sync.dma_start(out=wv_t[:], in_=wvt_d[:, :])

    def load_chunk(eng, dram, ncols, col0, tag):
        t = loads.tile([P, 2, 512], F8, tag=tag, name="in_f8")
        ap = dram[:, :]
        src = bass.AP(tensor=ap.tensor, offset=col0,
                      ap=[[ncols, P], [P * ncols, 2], [1, 512]])
        eng.dma_start(out=t[:], in_=src)
        return t

    cond_c = [load_chunk(nc.sync, cond_d, NL, icc * 512, f"cond{icc}")
              for icc in range(NL // 512)]

    def rep4_ap(handle):
        ap = handle[:]
        return bass.AP(tensor=ap.tensor, offset=ap.offset,
                       ap=[[0, 4], [1, CK]])

    nc.scalar.dma_start(out=bq_c[:], in_=rep4_ap(bq_d))
    nc.scalar.dma_start(out=bk_c[:], in_=rep4_ap(bk_d))
    nc.scalar.dma_start(out=bv_b[:], in_=bcast_ap(bv_d, P, C))

    feat_c = [load_chunk(nc.gpsimd, feat_d, N, ncc * 512, f"feat{ncc}")
              for ncc in range(N // 512)]

    # PE warm-up bridge (HAM clock gate: ~3.4us to trip to 2.4 GHz).
    pe_filler(21)

    feat_res = persist.tile([P, 2, NL], F32)

    # Projection outputs; partitions 32..63 etc. hold replicas of 0..31.
    q_rep = persist.tile([P, NL], BF16)           # q[ck, i] x4 (energy moving)
    k_rep = persist.tile([P, N], BF16)            # k[ck, j] x4 (energy stationary)
    # vT[p, jpb, ko, c] = gamma * v[c, 2*(jpb*128+p)+ko]  (PV stationary)
    vT_sb = persist.tile([P, NJPB, 2, C], F8)

    # ---- q/k projections in 512-col chunks through the pv psum pool ----
    def qk_chunk(wt, bias_c, src_chunk, dst, dcol):
        ps = pvp.tile([P, 512], F32, tag="pv", name="qk_ps")
        for ct in range(2):
            nc.tensor.matmul(ps[:], wt[:, ct, :], src_chunk[:, ct, :],
                             start=(ct == 0), stop=(ct == 1))
        nc.vector.tensor_scalar(dst[:, dcol:dcol + 512], ps[:],
                                bias_c[:], None, op0=ADD)

    for icc in range(4):
        qk_chunk(wq_t, bq_c, cond_c[icc], q_rep, icc * 512)
    for ncc in range(4):
        qk_chunk(wk_t, bk_c, feat_c[ncc], k_rep, ncc * 512)
    # NOTE: all transient psum users (qk chunks, vt units, fillers) share
    # the pv pool's rotation and MUST be emitted before the first pv_step
    # allocates its long-lived accumulation tiles (gs >= LAG).

    # ---- vT projection unit: one (kb, ko) -> vT_sb[:, kb, ko, :] ----
    # stationary: feat columns kb*256 + 2m + ko (stride-2 select) so out
    # partition m holds key 2*(kb*128+m)+ko - the same pairing the energy
    # stationary uses, and the one the DR moving layout needs.
    def vt_unit(kb, ko):
        v_ps = pvp.tile([P, 512], F32, tag="pv", name="v_ps")
        ch, bl = divmod(kb, 2)
        for ct in range(2):
            col0 = bl * 256 + ko
            nc.tensor.matmul(
                v_ps[:, 0:C],
                feat_c[ch][:, ct, col0:bl * 256 + 256:2],
                wv_t[:, ct, :],
                start=(ct == 0), stop=(ct == 1))
        nc.vector.tensor_tensor(vT_sb[:, kb, ko, :],
                                v_ps[:, 0:C], bv_b[:], op=ADD)

    # ---- steady-state structures ----
    attnT_tiles = {}    # chunk -> [P, NJPB, 512, 2] fp8
    pv_tiles = {}       # chunk -> (cb0, cb1, den) psum tiles
    rcp_bs = {}

    def exp_step(gs):
        # step (ich, jpb): energy e_T for 256 keys (pair-selected) x the
        # chunk's 512 queries; exp writes the pair-interleaved attnT slab.
        ich, jpb = divmod(gs, NJPB)
        e_ps = ep.tile([P, 2, 512], F32, tag="e", name="e_ps")
        for ko in range(2):
            rr = 2 * (jpb % 2) + ko
            col0 = jpb * 256 + ko
            nc.tensor.matmul(
                e_ps[:, ko, :],
                k_rep[ts(rr, CK), col0:jpb * 256 + 256:2],
                q_rep[ts(rr, CK), ts(ich, 512)],
                start=True, stop=True, tile_position=(32 * rr, 0))
        if jpb == 0:
            attnT_tiles[ich] = attnTp.tile([P, NJPB, 512, 2], F8,
                                           tag="attnT", name="attnT")
        at = attnT_tiles[ich][:, jpb, :, :].rearrange("p i k -> p k i")
        nc.scalar.activation(at, e_ps[:], EXP)

    def pv_step(s):
        ich, jpb = divmod(s, NJPB)
        if jpb == 0:
            cb0 = pvp.tile([P, 512], F32, tag="pv", name="pv0")
            cb1 = pvp.tile([P, 512], F32, tag="pv", name="pv1")
            den = pvp.tile([P, 512], F32, tag="pv", name="pvd")
            pv_tiles[ich] = (cb0, cb1, den)
        cb0, cb1, den = pv_tiles[ich]
        mov = attnT_tiles[ich][:, jpb, :, :].rearrange("p i k -> p k i")
        for cb in range(2):
            nc.tensor.matmul(
                (cb0, cb1)[cb][:],
                vT_sb[:, jpb, :, ts(cb, P)],
                mov,
                start=(jpb == 0), stop=(jpb == NJPB - 1),
                perf_mode=mybir.MatmulPerfMode.DoubleRow)
        nc.tensor.matmul(
            den[0:1, :],
            ones8[:, :, 0:1],
            mov,
            start=(jpb == 0), stop=(jpb == NJPB - 1),
            perf_mode=mybir.MatmulPerfMode.DoubleRow)
        if jpb == NJPB - 1:
            finalize_chunk(ich)

    def finalize_chunk(ch):
        cb0, cb1, den = pv_tiles[ch]
        # denominators: psum [1, 512] -> SBUF -> 4 PE transposes (into the
        # den bank, now free) -> queries on partitions -> cheap reciprocal.
        den_sb = finp.tile([1, 512], F32, tag="densb")
        nc.vector.tensor_copy(den_sb[:], den[0:1, :])
        for b in range(4):
            nc.tensor.transpose(den[:, b:b + 1],
                                den_sb[0:1, ts(b, P)], ident[0:1, 0:1])
        rcp = finp.tile([P, 4], F32, tag="rcp")
        nc.vector.reciprocal(rcp[:], den[:, 0:4])
        rb = rcpbp.tile([P, 512], F32, tag="rcpb", name="rcp_b")
        rcp_bs[ch] = rb
        # hop through DRAM to reorder [p, b] -> column order b*128+p, then
        # a hwdge stride-0 broadcast DMA replicates it across partitions.
        rda = rcp_d[:]
        h1dst = bass.AP(tensor=rda.tensor, offset=ch * 512,
                        ap=[[1, P], [P, 4]])
        i1 = nc.gpsimd.dma_start(out=h1dst, in_=rcp[:])
        src = bass.AP(tensor=rda.tensor, offset=ch * 512,
                      ap=[[0, P], [1, 512]])
        i2 = nc.sync.dma_start(out=rb[:], in_=src)
        # explicit hop1 -> hop2 ordering: Tile does not track the DRAM
        # scratch tensor, and the two DMAs are on different queues.
        from concourse.instruction_name_ordered_set import (
            InstructionNameOrderedSet,
        )
        deps = InstructionNameOrderedSet()
        deps.add(i1.ins.name)
        i2.ins.add_sync_dependencies_from(deps)
        for cb in range(2):
            # plain copy first so the psum bank frees fast (the rotation's
            # next user would otherwise stall on the rcp chain's latency)
            stc = finp.tile([P, 512], F32, tag=f"stc{cb}")
            nc.vector.tensor_copy(stc[:], (cb0, cb1)[cb][:])
            st = finp.tile([P, 512], F32, tag=f"st{cb}")
            nc.vector.tensor_tensor(st[:], stc[:], rb[:], op=MULT)
            so = finp.tile([P, 512], F32, tag=f"so{cb}")
            nc.vector.tensor_tensor(so[:], st[:],
                                    feat_res[:, cb, ts(ch, 512)], op=ADD)
            nc.gpsimd.dma_start(out=out_d[ts(cb, P), ts(ch, 512)], in_=so[:])

    # ---- software pipeline ----
    nsteps = NCH * NJPB
    # all remaining projections emitted in the prologue: their psum tiles
    # must cycle the pv pool before the first pv_step allocation, and the
    # instructions simply stall on their input DMAs (program order is
    # what Tile's dependency semantics require - see the k-chunk NOTE).
    for ncc in range(4, 8):
        qk_chunk(wk_t, bk_c, feat_c[ncc], k_rep, ncc * 512)
    for kb in range(NJPB):
        for ko in range(2):
            vt_unit(kb, ko)
    for gs in range(nsteps + LAG):
        if gs == 16:
            for cb in range(2):
                nc.gpsimd.dma_start(out=feat_res[:, cb, :],
                                    in_=fres_d[ts(cb, P), :])
        if gs < nsteps:
            exp_step(gs)
        if gs >= LAG and gs - LAG < nsteps:
            pv_step(gs - LAG)


def _split_ctrl_waits(nc, cap=1):
    """Walrus in this image allows only ONE sync-wait command per
    instruction; Tile emits several on phase-boundary instructions (and one
    per live semaphore on the kernel-tail drain). Splitting the excess waits
    onto preceding same-engine NoOps is semantically identical (engine
    sequencers execute in order, so waiting on A then B == waiting on both)."""
    for fn in nc.m.functions:
        for bb in fn.blocks:
            insts = bb.instructions
            out = []
            changed = False
            for ins in insts:
                si = ins.sync_info
                if si is not None and si.on_wait and len(si.on_wait) > cap:
                    waits = list(si.on_wait)
                    for i, w in enumerate(waits[:-cap]):
                        nop = mybir.InstNoOp(
                            name=f"{ins.name}-w{i}",
                            engine=ins.engine,
                            ins=[], outs=[],
                            sync_info=mybir.SyncInfo(on_wait=[w], on_update=[]),
                        )
                        if hasattr(nc, "register_instruction"):
                            nc.register_instruction(nop, overwrite=True)
                        out.append(nop)
                    ins.sync_info = mybir.SyncInfo(
                        on_wait=waits[-cap:], on_update=list(si.on_update))
                    changed = True
                out.append(ins)
            if changed:
                insts[:] = out


def build_nc():
    nc = bass.Bass()
    with tile.TileContext(nc) as tc, ExitStack() as ctx:
        _emit(tc, ctx)
    _split_ctrl_waits(nc)
    return nc


def _prep_wt(w, rep):
    # [K, C] fp32 -> [128, 2, K*rep] bf16 with w_t[p, ct, r*K+k] = w[k, ct*128+p]
    import ml_dtypes
    wt = np.asarray(w, np.float32).T.reshape(2, P, -1)       # [ct, p, K]
    wt = np.transpose(wt, (1, 0, 2))                          # [p, ct, K]
    wt = np.tile(wt, (1, 1, rep))
    return np.ascontiguousarray(wt.reshape(P, -1).astype(ml_dtypes.bfloat16))


def make_in_maps(features, conditions, Wq, bq, Wk, bk, Wv, bv, gamma):
    import ml_dtypes
    feat = np.ascontiguousarray(np.asarray(features, np.float32).reshape(B, C, N))
    cond = np.ascontiguousarray(np.asarray(conditions, np.float32).reshape(B, C, N))
    feat_f8 = feat.astype(ml_dtypes.float8_e4m3fn)
    cond_f8 = cond.astype(ml_dtypes.float8_e4m3fn)
    g = np.float32(np.asarray(gamma, np.float32).reshape(()))
    wqt = _prep_wt(Wq, 4)
    wkt = _prep_wt(Wk, 4)
    wvt = _prep_wt(np.asarray(Wv, np.float32) * g, 1)
    bq_ = np.ascontiguousarray(np.asarray(bq, np.float32))
    bk_ = np.ascontiguousarray(np.asarray(bk, np.float32))
    bv_ = np.ascontiguousarray(np.asarray(bv, np.float32) * g)
    in_maps = []
    for core in range(NCORES):
        b, h = divmod(core, 2)
        n0 = h * NL
        in_maps.append({
            "feat": feat_f8[b],
            "cond": np.ascontiguousarray(cond_f8[b][:, n0:n0 + NL]),
            "fres": np.ascontiguousarray(feat[b][:, n0:n0 + NL]),
            "Wqt": wqt, "Wkt": wkt, "Wvt": wvt,
            "bq": bq_, "bk": bk_, "bv": bv_,
        })
    return in_maps


def kernel(features, conditions, Wq, bq, Wk, bk, Wv, bv, gamma):
    global LAST_EXEC_TIME_NS, LAST_TRACE
    in_maps = make_in_maps(features, conditions, Wq, bq, Wk, bk, Wv, bv, gamma)
    nc = build_nc()
    trace = os.environ.get("BASS_KERNEL_TRACE", "0") == "1"
    res = run_bass_kernel_spmd(nc, in_maps, list(range(NCORES)), trace=trace)
    LAST_EXEC_TIME_NS = res.exec_time_ns
    LAST_TRACE = res.instructions_and_trace
    out = np.empty((B, C, N), np.float32)
    for core in range(NCORES):
        b, h = divmod(core, 2)
        out[b][:, h * NL:(h + 1) * NL] = res.results[core]["out"]
    return out.reshape(B, C, H, W)
